# revision 21
# baseline (speedup 1.0000x reference)
"""Trainium2 Bass kernel for a pre-LN multi-head attention block.

Computes, for x of shape (4, 2048, 512):
    xn  = LayerNorm(x) * gamma + beta
    q/k/v = xn @ W{q,k,v}.T + b{q,k,v}          (8 heads, dk=64)
    attn  = softmax(q k^T / sqrt(dk)) @ v
    out   = attn @ Wo.T + bo

Sharding: 8 cores = (4 batches) x (2 query-halves). Every core computes
LayerNorm + K/V for its batch's full 2048-token sequence and Q only for
its 1024 queries, so per-core outputs are disjoint row blocks of the
final result and the host gather is pure concatenation (no reduction).
The SPMD program is identical on all cores; per-core differences are
data-only (each core's x is passed with its query rows first -- attention
is invariant to key ordering as long as K and V share it).

On-chip layout (per core): everything is kept transposed,
scores^T[key, query], so the softmax reduction lands on the PE via a
ones-column appended to V (row 64 of the PV accumulator = softmax
denominators), and exp() is the only ScalarE pass over the n^2 scores.
Matmul operands are bf16 (fp32 streams at half rate through the PE);
all accumulation is fp32 in PSUM.

The attention inner loop is ScalarE(exp)-paced, which leaves the PE
~10% idle per key tile -- enough for the PE_HAM activity monitor to
clock-gate the PE to half rate, where it saturates and never recovers
(measured: K=4/8 for 200us+). To keep the PE dense we interleave the
Q/K projections for later heads plus, once those run out, dummy
matmuls into the attention loop as filler work.
"""

import ml_dtypes
import numpy as np

import concourse.bass as bass
import concourse.mybir as mybir
import concourse.tile as tile
from concourse import bacc
from concourse.bass_utils import run_bass_kernel_spmd
from concourse.masks import make_identity

F32 = mybir.dt.float32
BF16 = mybir.dt.bfloat16
ALU = mybir.AluOpType
ACTF = mybir.ActivationFunctionType

P = 128          # partitions
DIM = 512        # model dim
H = 8            # heads
DK = 64          # head dim
NTOK = 2048      # tokens per core (one batch's sequence)
NQ = 1024        # queries per core (half the sequence)
CC = DIM // P    # 4 contraction chunks of 128
TT = NTOK // P   # 16 token tiles
JT = NTOK // P   # 16 key tiles
NB = 512         # moving-operand limit per matmul
EPS = 1e-5
SCALE = DK ** -0.5

# Filler spacing: one filler unit every FILL_EVERY key tiles.
FILL_EVERY = 2

N_CORES = 8
_BUILT = None


def _build():
    nc = bacc.Bacc("TRN2", target_bir_lowering=False, debug=False,
                   num_devices=N_CORES)

    xq = nc.dram_tensor("xq", [NTOK, DIM], F32, kind="ExternalInput")
    wqT = nc.dram_tensor("wqT", [DIM, DIM], BF16, kind="ExternalInput")
    wkT = nc.dram_tensor("wkT", [DIM, DIM], BF16, kind="ExternalInput")
    wvT = nc.dram_tensor("wvT", [DIM, DIM], BF16, kind="ExternalInput")
    woT = nc.dram_tensor("woT", [DK, H, DIM], BF16, kind="ExternalInput")
    qb_c = nc.dram_tensor("qb_c", [P, CC], F32, kind="ExternalInput")
    kb_c = nc.dram_tensor("kb_c", [P, CC], F32, kind="ExternalInput")
    gam_c = nc.dram_tensor("gam_c", [P, CC], F32, kind="ExternalInput")
    bet_c = nc.dram_tensor("bet_c", [P, CC], F32, kind="ExternalInput")
    bv_b = nc.dram_tensor("bv_b", [P, DIM], F32, kind="ExternalInput")
    bo_b = nc.dram_tensor("bo_b", [P, DIM], F32, kind="ExternalInput")
    y = nc.dram_tensor("y", [NQ, DIM], F32, kind="ExternalOutput")

    with tile.TileContext(nc) as tc:
        with (
            tc.tile_pool(name="const", bufs=1) as const,
            tc.tile_pool(name="persist", bufs=1) as persist,
            tc.tile_pool(name="lnp", bufs=6) as lnp,
            tc.tile_pool(name="stp", bufs=8) as stp,
            tc.tile_pool(name="epp", bufs=3) as epp,
            tc.tile_pool(name="otp", bufs=2) as otp,
            tc.tile_pool(name="rpp", bufs=4) as rpp,
            tc.tile_pool(name="outp", bufs=3) as outp,
            # PSUM: 4 banks (s) + 2 banks (ops) + 2 banks (work) = 8
            tc.tile_pool(name="spp", bufs=2, space="PSUM") as spp,
            tc.tile_pool(name="opp", bufs=1, space="PSUM") as opp,
            tc.tile_pool(name="wpp", bufs=2, space="PSUM") as wpp,
        ):
            # x tile loads first -- LayerNorm is the head of the critical
            # path; weights are not needed until the projections.
            ident = const.tile([P, P], BF16)
            make_identity(nc, ident)
            xts = []
            for tt in range(TT):
                xt = lnp.tile([P, DIM], F32, tag="xt", name=f"xt{tt}")
                nc.sync.dma_start(out=xt, in_=xq.ap()[tt * P:(tt + 1) * P, :])
                xts.append(xt)

            qb = const.tile([P, CC], F32)
            nc.sync.dma_start(out=qb, in_=qb_c.ap())
            kb = const.tile([P, CC], F32)
            nc.sync.dma_start(out=kb, in_=kb_c.ap())
            gam = const.tile([P, CC], F32)
            nc.sync.dma_start(out=gam, in_=gam_c.ap())
            bet = const.tile([P, CC], F32)
            nc.sync.dma_start(out=bet, in_=bet_c.ap())
            bvb = const.tile([P, DIM], F32)
            nc.sync.dma_start(out=bvb, in_=bv_b.ap())
            bob = const.tile([P, DIM], F32)
            nc.sync.dma_start(out=bob, in_=bo_b.ap())
            epst = const.tile([P, 1], F32)
            nc.vector.memset(epst, EPS)
            wv = const.tile([P, CC, DIM], BF16)
            nc.sync.dma_start(out=wv, in_=wvT.ap().rearrange(
                "(cc p) d -> p cc d", p=P))
            wq = const.tile([P, CC, DIM], BF16)
            nc.sync.dma_start(out=wq, in_=wqT.ap().rearrange(
                "(cc p) d -> p cc d", p=P))
            wk = const.tile([P, CC, DIM], BF16)
            nc.sync.dma_start(out=wk, in_=wkT.ap().rearrange(
                "(cc p) d -> p cc d", p=P))
            wo = const.tile([DK, H, DIM], BF16)
            nc.sync.dma_start(out=wo, in_=woT.ap())

            # Persistent activations (alive across phases).
            xnT = persist.tile([P, CC, NTOK], BF16)    # xn^T; chunk cc = dims [128cc,..)
            qt = persist.tile([P, CC, NQ], BF16)       # Q^T; tile t = q-dims [128t,..)
            kt = persist.tile([P, CC, NTOK], BF16)     # K^T
            vp = persist.tile([P, JT, H, DK + 2], BF16)  # V' per key tile: [V_h | 1 | 0]
            onT = persist.tile([DK, H, NQ], BF16)      # normalized O^T per head

            nc.vector.memset(vp[:, :, :, DK], 1.0)
            nc.vector.memset(vp[:, :, :, DK + 1], 0.0)

            def qk_chunk(w, bias, dst, t, ib):
                # one (128, NB) psum chunk of the Q^T or K^T projection
                ps = wpp.tile([P, NB], F32, tag="w", name=f"qk{t}_{ib}_{w.name}")
                for cc in range(CC):
                    nc.tensor.matmul(ps, lhsT=w[:, cc, t * P:(t + 1) * P],
                                     rhs=xnT[:, cc, ib * NB:(ib + 1) * NB],
                                     start=(cc == 0), stop=(cc == CC - 1))
                nc.vector.tensor_scalar(
                    out=dst[:, t, ib * NB:(ib + 1) * NB], in0=ps,
                    scalar1=bias[:, t:t + 1], scalar2=None, op0=ALU.add)

            def v_proj(j):
                ps = wpp.tile([P, DIM], F32, tag="w", name=f"v{j}")
                for cc in range(CC):
                    nc.tensor.matmul(ps, lhsT=xnT[:, cc, j * P:(j + 1) * P],
                                     rhs=wv[:, cc, :],
                                     start=(cc == 0), stop=(cc == CC - 1))
                nc.vector.tensor_tensor(
                    out=vp[:, j, :, 0:DK],
                    in0=ps.rearrange("p (h d) -> p h d", d=DK),
                    in1=bvb.rearrange("p (h d) -> p h d", d=DK),
                    op=ALU.add)

            # ---- LayerNorm + transpose, with V and the first Q/K tile
            # pipelined in as token tiles complete.
            for tt in range(TT):
                xt = xts[tt]
                stats = stp.tile([P, 6], F32)
                nc.vector.bn_stats(out=stats, in_=xt)
                mv = stp.tile([P, 2], F32)
                nc.vector.bn_aggr(out=mv, in_=stats)
                rstd = stp.tile([P, 1], F32)
                nc.scalar.activation(out=rstd, in_=mv[:, 1:2], func=ACTF.Sqrt,
                                     bias=epst)
                nc.vector.reciprocal(out=rstd, in_=rstd)
                z = lnp.tile([P, DIM], BF16, tag="z")
                nc.vector.tensor_scalar(out=z, in0=xt, scalar1=mv[:, 0:1],
                                        scalar2=rstd, op0=ALU.subtract,
                                        op1=ALU.mult)
                for cc in range(CC):
                    ztp = wpp.tile([P, P], BF16, tag="w")
                    nc.tensor.transpose(ztp, z[:, cc * P:(cc + 1) * P], ident)
                    nc.vector.tensor_scalar(
                        out=xnT[:, cc, tt * P:(tt + 1) * P], in0=ztp,
                        scalar1=gam[:, cc:cc + 1], scalar2=bet[:, cc:cc + 1],
                        op0=ALU.mult, op1=ALU.add)
                v_proj(tt)
                if tt % 4 == 3:
                    ib = tt // 4
                    if ib < NQ // NB:
                        qk_chunk(wq, qb, qt, 0, ib)
                    qk_chunk(wk, kb, kt, 0, ib)

            # ---- Filler units: Q/K projections for later head pairs, then
            # dummy matmuls. One unit is consumed per FILL_EVERY key tiles
            # inside the attention loops to keep the PE dense (HAM-warm).
            filler = []
            for t in range(1, CC):
                for ib in range(NQ // NB):
                    filler.append(lambda t=t, ib=ib: qk_chunk(wq, qb, qt, t, ib))
                for ib in range(NTOK // NB):
                    filler.append(lambda t=t, ib=ib: qk_chunk(wk, kb, kt, t, ib))

            dummy_n = [0]

            def dummy(n=320):
                ps = wpp.tile([P, 512], F32, tag="w", name=f"d{dummy_n[0]}")
                dummy_n[0] += 1
                nc.tensor.matmul(ps[:, 0:n], lhsT=wv[:, 0, 0:P],
                                 rhs=wv[:, 0, 0:n], start=True, stop=True)

            def attention(h, deferred):
                hp, hm = divmod(h, 2)
                kt_h = kt[hm * DK:(hm + 1) * DK, hp, :]
                qt_h = qt[hm * DK:(hm + 1) * DK, hp, :]
                ops = opp.tile([DK + 2, NQ], F32, tag="ops", name=f"o{h}")
                for j in range(JT):
                    sps = spp.tile([P, NQ], F32, tag="sps", name=f"s{h}{j}")
                    for ib in range(NQ // NB):
                        nc.tensor.matmul(
                            sps[:, ib * NB:(ib + 1) * NB],
                            lhsT=kt_h[:, j * P:(j + 1) * P],
                            rhs=qt_h[:, ib * NB:(ib + 1) * NB],
                            start=True, stop=True)
                    et = epp.tile([P, NQ], BF16, tag="et", name=f"e{h}{j}")
                    nc.scalar.activation(out=et, in_=sps, func=ACTF.Exp,
                                         scale=SCALE)
                    if j == JT - 1:
                        dummy()
                        dummy()
                        dummy()
                    for ib in range(NQ // NB):
                        nc.tensor.matmul(
                            ops[:, ib * NB:(ib + 1) * NB],
                            lhsT=vp[:, j, h, :],
                            rhs=et[:, ib * NB:(ib + 1) * NB],
                            start=(j == 0), stop=(j == JT - 1))
                    if filler and j % 4 == 3:
                        filler.pop(0)()
                    else:
                        dummy()
                    if deferred and 1 <= j <= 8:
                        deferred.pop(0)()
                dummy()
                dummy()
                # Drain the PSUM accumulator fast; normalize is deferred in
                # 128-query chunks (a full-width DVE reciprocal is ~6us and
                # would block the in-order DVE queue, stalling the PSUM-copy
                # ops that release PE buffers).
                ot = otp.tile([DK + 1, NQ], F32, tag="ot", name=f"ot{h}")
                nc.vector.tensor_copy(out=ot, in_=ops[0:DK + 1, :])
                rsum = rpp.tile([1, NQ], F32, tag="r", name=f"r{h}")
                rb = rpp.tile([DK, NQ], F32, tag="rb", name=f"rb{h}")

                def norm_chunk(i, h=h, ot=ot, rsum=rsum, rb=rb):
                    cs = slice(i * P, (i + 1) * P)
                    nc.vector.reciprocal(out=rsum[:, cs], in_=ot[DK:DK + 1, cs])
                    nc.gpsimd.partition_broadcast(rb[:, cs], rsum[:, cs])
                    nc.vector.tensor_tensor(out=onT[:, h, i * P:(i + 1) * P],
                                            in0=ot[0:DK, cs], in1=rb[:, cs],
                                            op=ALU.mult)
                return [lambda i=i: norm_chunk(i) for i in range(NQ // P)]

            def y_proj(it):
                yps = wpp.tile([P, DIM], F32, tag="w", name=f"y{it}")
                for h in range(H):
                    nc.tensor.matmul(
                        yps, lhsT=onT[:, h, it * P:(it + 1) * P],
                        rhs=wo[:, h, :],
                        start=(h == 0), stop=(h == H - 1))
                yo = outp.tile([P, DIM], F32)
                nc.vector.tensor_tensor(out=yo, in0=yps, in1=bob, op=ALU.add)
                nc.sync.dma_start(out=y.ap()[it * P:(it + 1) * P, :], in_=yo)

            deferred = []
            for h in range(H):
                deferred = attention(h, deferred)
            # last head: pipeline each normalize chunk with its Y tile,
            # with dummies covering the drain latency so the PE stays warm
            for _ in range(4):
                dummy()
            for it in range(NQ // P):
                deferred.pop(0)()
                y_proj(it)

    nc.compile()
    return nc


def _get_nc():
    global _BUILT
    if _BUILT is None:
        _BUILT = _build()
    return _BUILT


def prep_in_maps(inputs):
    x = np.asarray(inputs["x"], np.float32)
    B, N, D = x.shape
    assert (B, N, D) == (4, 2048, 512)

    def cols(v):  # (512,) -> (128, 4): column t = v[128t:128(t+1)]
        return np.ascontiguousarray(np.asarray(v, np.float32).reshape(CC, P).T)

    def bcast(v):  # (512,) -> (128, 512)
        return np.ascontiguousarray(
            np.broadcast_to(np.asarray(v, np.float32), (P, DIM)))

    bf16 = ml_dtypes.bfloat16
    common = {
        "wqT": np.ascontiguousarray(np.asarray(inputs["Wq"], np.float32).T
                                    .astype(bf16)),
        "wkT": np.ascontiguousarray(np.asarray(inputs["Wk"], np.float32).T
                                    .astype(bf16)),
        "wvT": np.ascontiguousarray(np.asarray(inputs["Wv"], np.float32).T
                                    .astype(bf16)),
        "woT": np.ascontiguousarray(
            np.asarray(inputs["Wo"], np.float32).T
            .reshape(H, DK, DIM).transpose(1, 0, 2).astype(bf16)),
        "qb_c": cols(inputs["bq"]), "kb_c": cols(inputs["bk"]),
        "gam_c": cols(inputs["ln_gamma"]), "bet_c": cols(inputs["ln_beta"]),
        "bv_b": bcast(inputs["bv"]), "bo_b": bcast(inputs["bo"]),
    }
    in_maps = []
    for c in range(N_CORES):
        b, half = divmod(c, 2)
        o = half * NQ
        xc = np.concatenate([x[b, o:o + NQ], x[b, NQ - o:N - o]], axis=0)
        in_maps.append({"xq": np.ascontiguousarray(xc), **common})
    return in_maps


def kernel(x, ln_gamma, ln_beta, Wq, bq, Wk, bk, Wv, bv, Wo, bo):
    in_maps = prep_in_maps(dict(
        x=x, ln_gamma=ln_gamma, ln_beta=ln_beta, Wq=Wq, bq=bq, Wk=Wk, bk=bk,
        Wv=Wv, bv=bv, Wo=Wo, bo=bo))

    nc = _get_nc()
    res = run_bass_kernel_spmd(nc, in_maps, core_ids=list(range(N_CORES)))

    B, N, D = 4, 2048, DIM
    out = np.empty((B, N, D), np.float32)
    for c in range(N_CORES):
        b, half = divmod(c, 2)
        o = half * NQ
        out[b, o:o + NQ] = res.results[c]["y"]
    return out
